# revision 29
# baseline (speedup 1.0000x reference)
"""Distance-encoded-bias multi-head self-attention on 8 Trainium2 NeuronCores.

Strategy
--------
Shard (batch b in 0..1) x (head-pair in 0..3) -> 8 cores. Each core computes
its 2 heads' full attention for its batch and ships the UN-normalized
attention output plus the softmax denominator row ([65, N] per head); the
host divides, concatenates the 8 heads and applies the output projection
(one [N,512]@[512,512] matmul per batch).

Key algebraic moves (all exact):
 * Tokens are sorted by coordinate on the host (attention is permutation
   equivariant; output rows are inverse-permuted back).
 * cos(w|xi-xj|) = C_i C_j + S_i S_j with C=cos(w x), S=sin(w x), and
   sin(w|xi-xj|) = sign(xi-xj)(S_i C_j - C_i S_j). With sorted coords the
   sign is uniform per (key-chunk, query-range): each 512-wide half is
   computed with ONE full 97-row matmul using a fixed sign sigma; on the
   diagonal half the wrong-sign region is patched with a single 256-wide
   rank-16 matmul carrying 2x-scaled features, and the 128-wide diagonal
   window is patched by a host-precomputed additive block (dwin) that also
   absorbs the sigma choice.
 * The Gaussian local term E = exp(-d^2/ell^2) is built on device from a
   rank-3 factorization of d^2 + exp, and added into the score PSUM by
   ta_h-scaled identity matmuls.
 * Softmax uses no shift (scores are O(10) here); the denominator comes
   from a ones-column appended to V and is divided out on the host.
 * V is computed transposed (vdims on partitions, F=1024 matmuls at full
   rate) then flipped back with cheap PE transposes.
 * Matmul operands are fp32 bitcast to float32r (full-rate rows vs 1/4 for
   fp32); PSUM accumulation stays fp32.
"""

import math

import numpy as np

B, N, DIM, H, NF = 2, 1024, 512, 8, 8
HD = DIM // H
SCALE = HD ** -0.5
NCORES = 8
CHUNK = 128
NCHUNKS = N // CHUNK

_PROGRAM_CACHE = {}


def _bf16():
    import ml_dtypes

    return ml_dtypes.bfloat16


def _softplus64(x):
    x = np.asarray(x, np.float64)
    return np.log1p(np.exp(-np.abs(x))) + np.maximum(x, 0.0) + 1e-12


def _split_excess_waits(nc, max_waits=1):
    """CoreV3 walrus allows only one sync-wait command on some instruction
    encodings; move excess waits onto preceding same-engine NoOps."""
    import concourse.mybir as mybir
    import bass_rust

    n_split = 0
    for bb in nc.main_func.blocks:
        new_list = []
        changed = False
        for ins in bb.instructions:
            si = ins.sync_info
            waits = list(si.on_wait) if (si and si.on_wait) else []
            if len(waits) > max_waits:
                changed = True
                extra, keep = waits[:-max_waits], waits[-max_waits:]
                for i in range(0, len(extra), max_waits):
                    chunk = extra[i : i + max_waits]
                    n_split += 1
                    new_list.append(
                        mybir.InstNoOp(
                            name=f"{ins.name}-ws{i}",
                            engine=ins.engine,
                            ins=[],
                            outs=[],
                            sync_info=bass_rust.SyncInfo(
                                on_wait=chunk, on_update=[]
                            ),
                        )
                    )
                si.on_wait = keep
            new_list.append(ins)
        if changed:
            bb.instructions = new_list
    return n_split


def _build_program(biases_zero=True, repeat=1):
    key = ("nc", biases_zero, repeat)
    if key in _PROGRAM_CACHE:
        return _PROGRAM_CACHE[key]

    import concourse.bass as bass
    import concourse.mybir as mybir
    import concourse.tile as tile

    f32 = mybir.dt.float32
    f32r = mybir.dt.float32r
    bf16 = mybir.dt.bfloat16
    Alu = mybir.AluOpType
    Act = mybir.ActivationFunctionType

    nc = bass.Bass(trn_type="TRN2")

    # ---- per-core DRAM I/O ------------------------------------------------
    xt_d = nc.dram_tensor("xt", [DIM, N], bf16, kind="ExternalInput")
    identx_d = nc.dram_tensor("identx", [128, 384], f32r, kind="ExternalInput")
    wqk_d = nc.dram_tensor("wqk", [DIM, 256], bf16, kind="ExternalInput")
    wv_d = nc.dram_tensor("wv", [DIM, 128], bf16, kind="ExternalInput")
    qb_d = nc.dram_tensor("qb", [2, HD, 1], f32, kind="ExternalInput")
    kb_d = nc.dram_tensor("kb", [2, HD, 1], f32, kind="ExternalInput")
    # 33 rows: C,S feats (16) | zeros (1) | sin-side combos (16)
    kext_d = nc.dram_tensor("kext", [2, 33, N], f32r, kind="ExternalInput")
    qextp_d = nc.dram_tensor("qextp", [2, 33, N], f32r, kind="ExternalInput")
    qextm_d = nc.dram_tensor("qextm", [2, 33, N], f32r, kind="ExternalInput")
    # column-packed [2qc | -2qc | feat], each N wide
    qcs_d = nc.dram_tensor("qcs", [2, 16, 3 * N], f32r, kind="ExternalInput")
    erl_d = nc.dram_tensor("erl", [3, N], f32r, kind="ExternalInput")
    errr_d = nc.dram_tensor("errr", [3, 8 * N], f32r, kind="ExternalInput")
    dwin_d = nc.dram_tensor("dwin", [128, 16 * 128], f32r, kind="ExternalInput")
    tasc_d = nc.dram_tensor("tasc", [128, 2], f32, kind="ExternalInput")
    yt_d = nc.dram_tensor("yt", [2, 65, N], f32, kind="ExternalOutput")

    with tile.TileContext(nc) as tc:
      for _rep in range(repeat):
        with (
            tc.tile_pool(name="persist", bufs=1) as pers,
            tc.tile_pool(name="work", bufs=3) as work,
            tc.tile_pool(name="yg", bufs=2) as ygp,
        ):
            # ---- persistent SBUF tiles + input DMA, issued in the order the
            # prolog consumes them (head-0 features first, x/weights, head-1)
            def pt(shape, tag, dt=f32):
                return pers.tile(shape, dt, tag=tag, name=tag)

            erl_t = pt([3, N], "erl", f32r)
            nc.sync.dma_start(erl_t[:], erl_d[:])
            errr_t = pt([3, 8 * N], "errr", f32r)
            nc.sync.dma_start(errr_t[:], errr_d[:])
            e_t = [pt([128, N], f"e{k}", f32r) for k in range(NCHUNKS)]

            kf_t = [pt([97, N], f"kf{h}", f32r) for h in range(2)]
            qap_t = [pt([97, N], f"qap{h}", f32r) for h in range(2)]
            qam_t = [pt([97, N], f"qam{h}", f32r) for h in range(2)]
            qcs_t, qc2_t, qc2n_t, feat_t = [], [], [], []
            dwin_t = pt([128, 16 * 128], "dwin", f32r)
            identx_t = pt([128, 384], "identx", f32r)
            for h in range(2):
                s = pt([16, 3 * N], f"qcs{h}", f32r)
                qcs_t.append(s)
                qc2_t.append(s[:, 0:N])
                qc2n_t.append(s[:, N : 2 * N])
                feat_t.append(s[:, 2 * N : 3 * N])
            # head-0 score-side features first
            h = 0
            nc.sync.dma_start(kf_t[h][64:97, :], kext_d[h])
            nc.sync.dma_start(qap_t[h][64:97, :], qextp_d[h])
            nc.sync.dma_start(qam_t[h][64:97, :], qextm_d[h])
            nc.sync.dma_start(qcs_t[h][:], qcs_d[h])
            nc.sync.dma_start(dwin_t[:, 0 : 8 * 128], dwin_d[:, 0 : 8 * 128])
            nc.sync.dma_start(identx_t[:], identx_d[:])
            tasc_t = pt([128, 2], "tasc")
            nc.sync.dma_start(tasc_t[:], tasc_d[:])
            identa_t = [identx_t[:, 0:128], identx_t[:, 128:256]]
            identr_t = identx_t[:, 256:384]

            wqk_t = []
            for kc in range(4):
                s = pt([128, 256], f"wqk{kc}", bf16)
                nc.sync.dma_start(s[:], wqk_d[kc * 128 : (kc + 1) * 128, :])
                wqk_t.append(s)
            wv_t = []
            for kc in range(4):
                s = pt([128, 128], f"wv{kc}", bf16)
                nc.sync.dma_start(s[:], wv_d[kc * 128 : (kc + 1) * 128, :])
                wv_t.append(s)
            xT_t = []
            for c in range(4):
                s = pt([128, N], f"xT{c}", bf16)
                nc.sync.dma_start(s[:], xt_d[c * 128 : (c + 1) * 128, :])
                xT_t.append(s)
            # head-1 score-side features
            h = 1
            nc.sync.dma_start(kf_t[h][64:97, :], kext_d[h])
            nc.sync.dma_start(qap_t[h][64:97, :], qextp_d[h])
            nc.sync.dma_start(qam_t[h][64:97, :], qextm_d[h])
            nc.sync.dma_start(qcs_t[h][:], qcs_d[h])
            nc.sync.dma_start(dwin_t[:, 8 * 128 :], dwin_d[:, 8 * 128 :])

            qb_t, kb_t = [], []
            if not biases_zero:
                for h in range(2):
                    s = pt([HD, 1], f"qb{h}")
                    nc.sync.dma_start(s[:], qb_d[h])
                    qb_t.append(s)
                    s = pt([HD, 1], f"kb{h}")
                    nc.sync.dma_start(s[:], kb_d[h])
                    kb_t.append(s)

            vT_s = pt([128, N], "vTs", f32r)
            # per token-chunk: [ones | h0 vals 64 | ones | h1 vals 64]; the
            # ones columns land the softmax denominator in output row 0
            vo2_t = [pt([128, 130], f"vo{t}", f32r) for t in range(8)]

            # ---- prolog: qk^T for both heads + v^T, interleaved per k-chunk
            with (
                tc.tile_pool(name="ppro", bufs=2, space="PSUM") as ppro,
                tc.tile_pool(name="pvt", bufs=1, space="PSUM") as pvtp,
                tc.tile_pool(name="pe2", bufs=2, space="PSUM") as pep,
            ):
                # E(k) = exp(-d^2/ell^2): d^2 is rank-3 in the sorted coords.
                for k in range(NCHUNKS):
                    j0 = k * 128
                    for nh in range(2):
                        pe2 = pep.tile([128, 512], f32, tag="pe2")
                        nc.tensor.matmul(
                            pe2[:],
                            lhsT=erl_t[:, j0 : j0 + 128],
                            rhs=errr_t[:, k * N + nh * 512 : k * N + (nh + 1) * 512],
                            start=True, stop=True, skip_group_check=True,
                        )
                        nc.scalar.activation(
                            e_t[k][:, nh * 512 : (nh + 1) * 512],
                            pe2[:], Act.Exp,
                        )

                pqk = [
                    ppro.tile([128, N], f32, tag="ppro", name=f"pqk{i}")
                    for i in range(2)
                ]
                pvt = pvtp.tile([128, N], f32, tag="pvt")
                for kc in range(4):
                    for h in range(2):
                        for nh in range(2):
                            nc.tensor.matmul(
                                pqk[h][:, nh * 512 : (nh + 1) * 512],
                                lhsT=wqk_t[kc][:, h * 128 : (h + 1) * 128],
                                rhs=xT_t[kc][:, nh * 512 : (nh + 1) * 512],
                                start=(kc == 0),
                                stop=(kc == 3),
                            )
                    for nh in range(2):
                        nc.tensor.matmul(
                            pvt[:, nh * 512 : (nh + 1) * 512],
                            lhsT=wv_t[kc][:],
                            rhs=xT_t[kc][:, nh * 512 : (nh + 1) * 512],
                            start=(kc == 0),
                            stop=(kc == 3),
                        )

                # ones columns of every vo tile (cheap, during the DMA window)
                for t in range(8):
                    nc.vector.memset(vo2_t[t][:, 0:1].bitcast(f32), 1.0)
                    nc.vector.memset(vo2_t[t][:, 65:66].bitcast(f32), 1.0)

                # score-side fills; q rows carry SCALE. Head-0 fills gate the
                # main loop; head-1 DVE pieces are deferred into the h0 loop
                def fill_qap(h):
                    p = pqk[h]
                    for nh in range(2):
                        cs_ = slice(nh * 512, (nh + 1) * 512)
                        if biases_zero:
                            nc.scalar.mul(
                                qap_t[h][0:64, cs_], p[0:64, cs_], SCALE
                            )
                        else:
                            nc.vector.tensor_scalar(
                                qap_t[h][0:64, cs_], p[0:64, cs_],
                                scalar1=qb_t[h][:], scalar2=SCALE,
                                op0=Alu.add, op1=Alu.mult,
                            )

                def fill_piece(h, i):
                    p = pqk[h]
                    nh = i // 2
                    cs_ = slice(nh * 512, (nh + 1) * 512)
                    if i % 2 == 0:
                        if biases_zero:
                            nc.vector.tensor_scalar_mul(
                                qam_t[h][0:64, cs_], p[0:64, cs_], SCALE
                            )
                        else:
                            nc.vector.tensor_scalar(
                                qam_t[h][0:64, cs_], p[0:64, cs_],
                                scalar1=qb_t[h][:], scalar2=SCALE,
                                op0=Alu.add, op1=Alu.mult,
                            )
                    else:
                        if biases_zero:
                            nc.vector.tensor_copy(
                                kf_t[h][0:64, cs_], p[64:128, cs_]
                            )
                        else:
                            nc.vector.tensor_scalar(
                                kf_t[h][0:64, cs_], p[64:128, cs_],
                                scalar1=kb_t[h][:], scalar2=None, op0=Alu.add,
                            )

                def fill_piece_act(h, i):
                    p = pqk[h]
                    nh = i // 2
                    cs_ = slice(nh * 512, (nh + 1) * 512)
                    if i % 2 == 0:
                        nc.scalar.mul(
                            qam_t[h][0:64, cs_], p[0:64, cs_], SCALE
                        )
                    else:
                        nc.scalar.mul(
                            kf_t[h][0:64, cs_], p[64:128, cs_], 1.0
                        )

                fill_qap(0)
                for i in range(4):
                    fill_piece(0, i)
                # v^T -> SBUF (DVE); head-1 score-side fills on Act (their
                # first use is ~13us into the main loop)
                for nh in range(2):
                    cs_ = slice(nh * 512, (nh + 1) * 512)
                    nc.vector.tensor_copy(vT_s[:, cs_], pvt[:, cs_])
                fill_qap(1)
                if biases_zero:
                    for i in range(4):
                        fill_piece_act(1, i)
                else:
                    for i in range(4):
                        fill_piece(1, i)

            # ---- main attention loop (attn@V pipelined one chunk back) ----
            with (
                tc.tile_pool(name="pp", bufs=3, space="PSUM") as ppp,
                tc.tile_pool(name="po", bufs=1, space="PSUM") as pop,
            ):
                def scores_chunk(h, k, p):
                    j0 = k * 128
                    ch = slice(j0, j0 + 128)
                    sig_p = (k % 4) in (0, 2)
                    dh0 = (k // 4) * 512        # half containing the chunk
                    fh0 = 512 - dh0             # far half
                    # DVE pre-writes ta_h*E (+dwin on the 128-wide window)
                    # into the diagonal half FIRST so PE work overlaps it
                    nc.vector.scalar_tensor_tensor(
                        p[:, ch], e_t[k][:, ch], tasc_t[:, h : h + 1],
                        dwin_t[:, (h * 8 + k) * 128 : (h * 8 + k + 1) * 128],
                        op0=Alu.mult, op1=Alu.add,
                    )
                    if j0 > dh0:
                        nc.vector.tensor_scalar_mul(
                            p[:, dh0:j0], e_t[k][:, dh0:j0],
                            tasc_t[:, h : h + 1],
                        )
                    if j0 + 128 < dh0 + 512:
                        nc.vector.tensor_scalar_mul(
                            p[:, j0 + 128 : dh0 + 512],
                            e_t[k][:, j0 + 128 : dh0 + 512],
                            tasc_t[:, h : h + 1],
                        )
                    # far half: init with ta_h*E via identity matmul, then
                    # one single-sign K=97 matmul
                    fcols = slice(fh0, fh0 + 512)
                    nc.tensor.matmul(
                        p[:, fcols], lhsT=identa_t[h],
                        rhs=e_t[k][:, fcols],
                        start=True, stop=False, skip_group_check=True,
                    )
                    src = qam_t[h] if j0 > fh0 else qap_t[h]
                    nc.tensor.matmul(
                        p[:, fcols], lhsT=kf_t[h][:, ch],
                        rhs=src[:, fcols],
                        start=False, stop=True, skip_group_check=True,
                    )
                    # diagonal half: full-width single-sign matmul on top of
                    # the DVE init; wrong-sign region patched with one
                    # 256-wide 2x-feature matmul
                    dcols = slice(dh0, dh0 + 512)
                    src = qap_t[h] if sig_p else qam_t[h]
                    nc.tensor.matmul(
                        p[:, dcols], lhsT=kf_t[h][:, ch],
                        rhs=src[:, dcols],
                        start=False, stop=(k % 4 in (0, 3)),
                        skip_group_check=True,
                    )
                    if k % 4 == 1:
                        # sigma=-1, right of window needs +: add +2qc
                        nc.tensor.matmul(
                            p[:, j0 + 128 : dh0 + 512],
                            lhsT=feat_t[h][:, ch],
                            rhs=qc2_t[h][:, j0 + 128 : dh0 + 512],
                            start=False, stop=True,
                            skip_group_check=True,
                        )
                    elif k % 4 == 2:
                        # sigma=+1, left of window needs -: add -2qc
                        nc.tensor.matmul(
                            p[:, dh0:j0],
                            lhsT=feat_t[h][:, ch],
                            rhs=qc2n_t[h][:, dh0:j0],
                            start=False, stop=True,
                            skip_group_check=True,
                        )
                    xb = work.tile([128, N], f32r, tag="xb")
                    nc.scalar.activation(xb[:], p[:], Act.Exp)
                    return xb

                def attnv_chunk(h, k, xb, o):
                    for nh in range(2):
                        nc.tensor.matmul(
                            o[0:65, nh * 512 : (nh + 1) * 512],
                            lhsT=vo2_t[k][:, h * 65 : (h + 1) * 65],
                            rhs=xb[:, nh * 512 : (nh + 1) * 512],
                            start=(k == 0),
                            stop=(k == NCHUNKS - 1),
                            skip_group_check=True,
                        )

                # flip v^T back: PE transposes into one long-lived PSUM tile,
                # vo copies (DVE) trickled through the h=0 loop
                tpall = ppp.tile([128, N], f32, tag="pp", name="tpall")
                tprs = []
                for t in range(8):
                    tpr = tpall[:, t * 128 : (t + 1) * 128].bitcast(f32r)
                    tprs.append(tpr)
                    nc.tensor.transpose(
                        tpr,
                        vT_s[:, t * 128 : (t + 1) * 128],
                        identr_t[:, :],
                    )

                def vo_fill(t):
                    nc.vector.tensor_copy(vo2_t[t][:, 1:65], tprs[t][:, 0:64])
                    nc.vector.tensor_copy(
                        vo2_t[t][:, 66:130], tprs[t][:, 64:128]
                    )

                vo_fill(0)
                for h in range(2):
                    o = pop.tile([128, N], f32, tag="po")
                    pending = None  # (k, xb) awaiting attn@V
                    for k in range(NCHUNKS):
                        p = ppp.tile([128, N], f32, tag="pp")
                        xb = scores_chunk(h, k, p)
                        if pending is not None:
                            attnv_chunk(h, pending[0], pending[1], o)
                        if h == 0 and k < NCHUNKS - 1:
                            vo_fill(k + 1)
                        pending = (k, xb)
                    attnv_chunk(h, pending[0], pending[1], o)

                    # ship un-normalized out + denominator row to the host
                    ob = ygp.tile([65, N], f32, tag="yg")
                    nc.vector.tensor_copy(ob[:, 0:512], o[0:65, 0:512])
                    nc.scalar.mul(ob[:, 512:1024], o[0:65, 512:1024], 1.0)
                    nc.sync.dma_start(yt_d[h], ob[:])

    _split_excess_waits(nc)
    _PROGRAM_CACHE[key] = nc
    return nc


def _prepare_in_maps(
    x_tokens, coords, qkv_w, qkv_b, proj_w, omega_raw, a, c,
    alpha_raw, ell_raw, bias_scale_raw,
):
    """Host-side preprocessing. Returns (in_maps, perms)."""
    x64 = np.asarray(x_tokens, np.float64)
    co64 = np.asarray(coords, np.float64)

    alpha = _softplus64(alpha_raw)            # (H,)
    ell = _softplus64(ell_raw)                # (H,)
    om = _softplus64(omega_raw)               # (H, F)
    t = np.tanh(np.asarray(bias_scale_raw, np.float64))  # (H,)
    a2 = t[:, None] * np.asarray(a, np.float64)          # (H, F)
    c2 = t[:, None] * np.asarray(c, np.float64)
    ta = t * alpha                                        # (H,)

    assert np.allclose(ell, ell[0]), "per-head ell not supported"

    ident = np.eye(128, dtype=np.float32)
    io, jo = np.meshgrid(np.arange(128), np.arange(128), indexing="ij")
    tri = np.sign(jo - io).astype(np.float64)  # TRI[p, c] = sign(c - p)

    perms, in_maps = [], []
    for b in range(B):
        perm = np.argsort(co64[b], kind="stable")
        perms.append(perm)
        cs = co64[b][perm]                      # sorted coords
        xs = x64[b][perm]                       # (N, DIM)
        l2 = ell[0] ** 2
        # per-key-chunk centering keeps the rank-3 d^2 factors small where
        # E = exp(-d^2/l^2) is non-negligible (f32r has ~12 mantissa bits)
        erl = np.empty((3, N))
        errr = np.empty((NCHUNKS, 3, N))
        for kk in range(NCHUNKS):
            mu = cs[kk * 128 : (kk + 1) * 128].mean()
            u = cs[kk * 128 : (kk + 1) * 128] - mu
            erl[:, kk * 128 : (kk + 1) * 128] = np.stack(
                [u**2, u, np.ones(128)]
            )
            w = cs - mu
            errr[kk] = np.stack([-np.ones(N) / l2, 2 * w / l2, -(w**2) / l2])
        erl = erl.astype(np.float32)
        # (NCHUNKS,3,N) -> (3, NCHUNKS*N) with chunk-k block at cols [k*N,(k+1)*N)
        errr = np.concatenate(list(errr), axis=1).astype(np.float32)

        for pair in range(4):
            heads = (2 * pair, 2 * pair + 1)
            wqk_cols, wv_cols = [], []
            qb_rows, kb_rows = [], []
            kext, qextp, qextm, qcs = [], [], [], []
            identa, dwin = [], []
            for h in heads:
                sl_q = slice(h * HD, (h + 1) * HD)
                sl_k = slice(DIM + h * HD, DIM + (h + 1) * HD)
                sl_v = slice(2 * DIM + h * HD, 2 * DIM + (h + 1) * HD)
                wqk_cols.append(np.asarray(qkv_w)[:, sl_q])
                wqk_cols.append(np.asarray(qkv_w)[:, sl_k])
                wv_cols.append(np.asarray(qkv_w)[:, sl_v])
                qb_rows.append(np.asarray(qkv_b)[sl_q])
                kb_rows.append(np.asarray(qkv_b)[sl_k])

                C = np.cos(om[h][:, None] * cs[None, :])   # (F, N)
                S = np.sin(om[h][:, None] * cs[None, :])
                kext.append(np.concatenate([C, S, np.ones((1, N)), C, S], axis=0))
                qa_rows = np.concatenate(
                    [a2[h][:, None] * C, a2[h][:, None] * S,
                     np.zeros((1, N))],
                    axis=0,
                )  # (17, N)
                qc_rows = np.concatenate(
                    [c2[h][:, None] * S, -c2[h][:, None] * C], axis=0
                )  # (16, N)
                qextp.append(np.concatenate([qa_rows, qc_rows], axis=0))
                qextm.append(np.concatenate([qa_rows, -qc_rows], axis=0))
                qcs.append(
                    np.concatenate(
                        [2 * qc_rows, -2 * qc_rows,
                         np.concatenate([C, S], axis=0)],
                        axis=1,
                    )
                )
                identa.append(np.eye(128) * ta[h])
                # diagonal 128x128 window patch: replace the uniform sigma
                # sign applied by the full-width matmul with the true
                # per-pair sign: dwin[k][j, i] = (tri[j,i]-sigma_k) * sp[j,i]
                featcs = np.concatenate([C, S], axis=0)      # (16, N)
                wins = []
                for k in range(NCHUNKS):
                    j0 = k * 128
                    sig = 1.0 if (k % 4) in (0, 2) else -1.0
                    blk = featcs[:, j0 : j0 + 128].T @ qc_rows[:, j0 : j0 + 128]
                    wins.append(blk * (tri - sig))
                dwin.append(np.stack(wins))

            in_maps.append(
                {
                    "xt": np.ascontiguousarray(xs.T.astype(_bf16())),
                    "identx": np.concatenate(
                        identa + [np.eye(128)], axis=1
                    ).astype(np.float32),
                    "wqk": np.ascontiguousarray(
                        np.concatenate(wqk_cols, axis=1).astype(_bf16())
                    ),
                    "wv": np.ascontiguousarray(
                        np.concatenate(wv_cols, axis=1).astype(_bf16())
                    ),
                    "qb": np.stack(qb_rows).astype(np.float32)[:, :, None],
                    "kb": np.stack(kb_rows).astype(np.float32)[:, :, None],
                    "kext": np.stack(kext).astype(np.float32),
                    "qextp": np.stack(qextp).astype(np.float32),
                    "qextm": np.stack(qextm).astype(np.float32),
                    "qcs": np.stack(qcs).astype(np.float32),
                    "erl": erl,
                    "errr": errr,
                    "dwin": np.concatenate(
                        [w for hw in dwin for w in hw], axis=1
                    ).astype(np.float32),
                    "tasc": np.tile(
                        ta[list(heads)].astype(np.float32)[None, :], (128, 1)
                    ),
                }
            )
    return in_maps, perms


def kernel(
    x_tokens, coords, qkv_w, qkv_b, proj_w, proj_b,
    omega_raw, a, c, alpha_raw, ell_raw, bias_scale_raw,
):
    from concourse.bass_utils import run_bass_kernel_spmd

    biases_zero = not np.any(np.asarray(qkv_b))
    nc = _build_program(biases_zero=biases_zero)
    in_maps, perms = _prepare_in_maps(
        x_tokens, coords, qkv_w, qkv_b, proj_w, omega_raw, a, c,
        alpha_raw, ell_raw, bias_scale_raw,
    )
    res = run_bass_kernel_spmd(nc, in_maps, core_ids=list(range(NCORES)))

    # v-bias contributes a constant row (attention weights sum to 1)
    vb = np.asarray(qkv_b, np.float64)[2 * DIM :]
    pw64 = np.asarray(proj_w, np.float64)
    const_row = vb @ pw64 + np.asarray(proj_b, np.float64)

    out = np.empty((B, N, DIM), np.float32)
    for b in range(B):
        OS = np.empty((DIM, N), np.float64)
        for pair in range(4):
            r = res.results[4 * b + pair]["yt"].astype(np.float64)  # (2,65,N)
            for hh in range(2):
                hg = 2 * pair + hh
                OS[hg * HD : (hg + 1) * HD] = r[hh, 1:65] / r[hh, 0:1]
        acc = OS.T @ pw64 + const_row[None, :]
        y = np.empty((N, DIM), np.float64)
        y[perms[b]] = acc
        out[b] = y.astype(np.float32)
    return out
